# revision 10
# baseline (speedup 1.0000x reference)
"""ANI-style AEVComputer on 8 TRN2 NeuronCores (Bass/Tile).

Strategy
--------
Data-parallel over conformations: each of the 8 cores processes 2 of the 16
conformations end to end; no cross-core communication.

Per conformation, on device:
  *  d^2 matrix via one 5-wide TensorE matmul  (A@B^T with A=[c,q,1], B=[-2c,1,q])
  *  d = exp(ln(d2)/2), cutoff fns via ACT Sin (arg folded into [-pi,pi])
  *  radial AEV: 16 shifted gaussians * fc, scattered over species by matmul
  *  angular AEV over atom pairs (j,k): all pair geometry is derived from the
     d/d^2/fc matrices by selection matmuls (rows I[p], J[p]); the
     (1+cos(theta-shfz))/2 ** 32 factor uses cos(theta-shfz) = c*cz + s*sz and
     pow via exp(32*ln(h)); scatter over the 10 species-pair bins by matmul
     with per-pair one-hot weights (PSUM-accumulated across pair chunks).

The host precomputes integer-derived selection/one-hot tables only (species
one-hots, pair index one-hots); all floating-point geometry math runs on
device.  Pairs (j,k) that can't contribute (no atom within the angular cutoff
of both) are compacted out on host; the kernel is compiled for the padded
live-pair count K (cached per K).
"""
import sys

if '/opt/trn_rl_repo' not in sys.path:
    sys.path.insert(0, '/opt/trn_rl_repo')

import numpy as np
import ml_dtypes

import concourse.bass as bass
import concourse.tile as tile
from concourse import mybir
from concourse.bass_utils import run_bass_kernel_spmd

DT = mybir.dt
AF = mybir.ActivationFunctionType
ALU = mybir.AluOpType

# ---------------- walrus compat: one sync wait per instruction ----------------


def _split_multiwaits(nc):
    n = 0
    for f in nc.m.functions:
        for bb in f.blocks:
            insts = bb.instructions
            out = []
            changed = False
            for inst in insts:
                si = inst.sync_info
                waits = list(si.on_wait) if si is not None else []
                if len(waits) > 1:
                    changed = True
                    for w in waits[:-1]:
                        n += 1
                        out.append(mybir.InstNoOp(
                            name=f"mwsplit-{n}", engine=inst.engine, ins=[], outs=[],
                            sync_info=mybir.SyncInfo(on_wait=[w], on_update=[]),
                        ))
                    inst.sync_info = mybir.SyncInfo(
                        on_wait=[waits[-1]], on_update=list(si.on_update))
                out.append(inst)
            if changed:
                insts.clear()
                insts.extend(out)
    return n


# ---------------- problem constants ----------------
RCR, RCA = 5.2, 3.5
ETA_R, ETA_A, ZETA = 16.0, 8.0, 32.0
SHF_R = (0.9 + 0.26875 * np.arange(16)).astype(np.float64)
SHF_A = np.array([0.9, 1.55, 2.2, 2.85], np.float64)
SHF_Z = (np.pi / 16 + (np.pi / 8) * np.arange(8)).astype(np.float64)
NSP = 4
C, A = 16, 64
NCORES, CPC = 8, 2            # cores, conformations per core
IDX_I, IDX_J = np.triu_indices(A, k=1)
P_FULL = IDX_I.size           # 2016
PCH = 126                     # pairs per chunk (partition dim of chunk tiles)

_tbl = np.zeros((NSP, NSP), np.int64)
_k = 0
for _a in range(NSP):
    for _b in range(_a, NSP):
        _tbl[_a, _b] = _tbl[_b, _a] = _k
        _k += 1
NPAIR_T = _k                  # 10

DIAG = RCR + 1.0              # value added on the diagonal of d

_NC_CACHE = {}
DEBUG_DUMP = False


def _build(K):
    """Build the per-core Bass graph for padded pair count K (multiple of PCH)."""
    CH = K // PCH
    nc = bass.Bass("TRN2", target_bir_lowering=False, debug=False)

    coords = nc.declare_dram_parameter("coords", [CPC, A, 3], DT.float32, isOutput=False)
    selit = nc.declare_dram_parameter("selit", [CPC, A, K], DT.float32, isOutput=False)
    seljt = nc.declare_dram_parameter("seljt", [CPC, A, K], DT.float32, isOutput=False)
    seljpf = nc.declare_dram_parameter("seljpf", [CPC, K, A], DT.float32, isOutput=False)
    ohp = nc.declare_dram_parameter("ohp", [CPC, K, NPAIR_T], DT.bfloat16, isOutput=False)
    ohs = nc.declare_dram_parameter("ohs", [CPC, A, NSP], DT.bfloat16, isOutput=False)
    out = nc.declare_dram_parameter("out", [CPC, A, 384], DT.float32, isOutput=True)
    dbg = None
    if DEBUG_DUMP:
        dbg = nc.declare_dram_parameter("dbg", [CPC, A, 4, A], DT.float32, isOutput=True)

    CZ = np.cos(SHF_Z)
    SZ = np.sin(SHF_Z)

    with tile.TileContext(nc) as tc:
        with tc.tile_pool(name="cpool", bufs=1) as cpool, \
             tc.tile_pool(name="sbC", bufs=2) as sbC, \
             tc.tile_pool(name="sbK", bufs=2) as sbK, \
             tc.tile_pool(name="ps1", bufs=1, space="PSUM") as ps1, \
             tc.tile_pool(name="psA", bufs=1, space="PSUM") as psA:

            consts = {}

            def cst(val):
                v = float(val)
                if v not in consts:
                    t = cpool.tile([128, 1], DT.float32, tag=f"cst{len(consts)}",
                                   name=f"cst{len(consts)}")
                    nc.vector.memset(t[:], v)
                    consts[v] = t
                return consts[v]

            # eye * DIAG via iota + is_equal
            iP = cpool.tile([A, A], DT.float32)
            nc.gpsimd.iota(iP[:], [[0, A]], channel_multiplier=1,
                           allow_small_or_imprecise_dtypes=True)
            iF = cpool.tile([A, A], DT.float32)
            nc.gpsimd.iota(iF[:], [[1, A]], channel_multiplier=0,
                           allow_small_or_imprecise_dtypes=True)
            eye = cpool.tile([A, A], DT.float32)
            nc.vector.tensor_tensor(eye[:], iP[:], iF[:], ALU.is_equal)
            nc.vector.tensor_scalar(eye[:], eye[:], DIAG, None, ALU.mult)
            onesrow = cpool.tile([1, A], DT.float32)
            nc.vector.memset(onesrow[:], 1.0)

            geos, fcRqs = [], []

            # ---------------- pass 1: per-conformation geometry ----------------
            for cc in range(CPC):
                # A9 = [c^2 ; 1 ; c],  B9 = [1 ; c^2 ; -2c]  ->  A9.T @ B9 = d^2
                A9 = sbC.tile([9, A], DT.float32, tag="A9", name=f"A9{cc}")
                B9 = sbC.tile([9, A], DT.float32, tag="B9", name=f"B9{cc}")
                ct = sbC.tile([3, A], DT.float32, tag="ct", name=f"ct{cc}")
                nc.sync.dma_start(ct[:], coords[cc].rearrange("a k -> k a"))
                nc.scalar.square(A9[0:3, :], ct[:])          # c^2   (parts 0-2)
                nc.vector.memset(B9[0:3, :], 1.0)             # ones (parts 0-2)
                m2ct = sbC.tile([3, A], DT.float32, tag="m2ct", name=f"m2ct{cc}")
                nc.vector.tensor_scalar(m2ct[:], ct[:], -2.0, None, ALU.mult)
                nc.sync.dma_start(A9[3:6, :], B9[0:3, :])     # ones
                nc.sync.dma_start(A9[6:9, :], coords[cc].rearrange("a k -> k a"))
                nc.sync.dma_start(B9[3:6, :], A9[0:3, :])     # c^2
                nc.sync.dma_start(B9[6:9, :], m2ct[:])        # -2c

                dsqp = ps1.tile([A, A], DT.float32, tag="dsq", name=f"dsq{cc}")
                nc.tensor.matmul(dsqp[:], A9[:], B9[:], start=True, stop=True)

                geo = sbC.tile([A, 3, A], DT.float32, tag="geo", name=f"geo{cc}")
                d_t, dsqc, fcA = geo[:, 0, :], geo[:, 1, :], geo[:, 2, :]
                nc.vector.tensor_scalar(dsqc, dsqp[:], 0.0, None, ALU.max)
                lnd = sbC.tile([A, A], DT.float32, tag="lnd", name=f"lnd{cc}")
                nc.scalar.activation(lnd[:], dsqc, AF.Ln)
                nc.scalar.activation(d_t, lnd[:], AF.Exp, scale=0.5)
                nc.vector.tensor_tensor(d_t, d_t, eye[:], ALU.add)

                # cutoffs: fc = mask * (0.5 + 0.5*sin(pi/2 - pi*d/rc))
                fcRq = sbC.tile([A, A], DT.float32, tag="fcRq", name=f"fcRq{cc}")
                for (dst, rc, s1, s2) in ((fcA, RCA, 0.5, 0.5), (fcRq, RCR, 0.125, 0.125)):
                    dcl = sbC.tile([A, A], DT.float32, tag="dcl", name=f"dcl{cc}{rc}")
                    nc.vector.tensor_scalar(dcl[:], d_t, rc * 1.01, None, ALU.min)
                    sn = sbC.tile([A, A], DT.float32, tag="sn", name=f"sn{cc}{rc}")
                    nc.scalar.activation(sn[:], dcl[:], AF.Sin,
                                         bias=cst(np.pi / 2)[:A, 0:1], scale=-np.pi / rc)
                    msk = sbC.tile([A, A], DT.float32, tag="msk", name=f"msk{cc}{rc}")
                    nc.vector.tensor_scalar(msk[:], d_t, rc, None, ALU.is_le)
                    nc.vector.tensor_scalar(sn[:], sn[:], s1, s2, ALU.mult, ALU.add)
                    nc.vector.tensor_tensor(dst, sn[:], msk[:], ALU.mult)

                if DEBUG_DUMP:
                    nc.sync.dma_start(dbg[cc, :, 0:3, :], geo[:])
                    nc.sync.dma_start(dbg[cc, :, 3:4, :],
                                      fcRq[:].rearrange("p (o i) -> p o i", o=1))
                geos.append(geo)
                fcRqs.append(fcRq)

            # ---------------- pass 2: radial + angular ----------------
            for cc in range(CPC):
                geo, fcRq = geos[cc], fcRqs[cc]
                d_t, dsqc, fcA = geo[:, 0, :], geo[:, 1, :], geo[:, 2, :]

                # radial
                rt = sbC.tile([A, 16, A], DT.float32, tag="rt", name=f"rt{cc}")
                for r in range(16):
                    nc.scalar.activation(rt[:, r, :], d_t, AF.Square,
                                         bias=cst(-SHF_R[r])[:A, 0:1])
                nc.scalar.activation(rt[:], rt[:], AF.Exp, scale=-ETA_R)
                rtm = sbC.tile([A, 16, A], DT.bfloat16, tag="rtm", name=f"rtm{cc}")
                nc.vector.tensor_tensor(
                    rtm[:], rt[:],
                    fcRq[:].rearrange("p (r i) -> p r i", r=1).broadcast_to([A, 16, A]),
                    ALU.mult)
                ohs_sb = sbC.tile([A, NSP], DT.bfloat16, tag="ohs", name=f"ohs{cc}")
                nc.sync.dma_start(ohs_sb[:], ohs[cc])
                radsb = sbC.tile([NSP, A, 16], DT.float32, tag="radsb", name=f"radsb{cc}")
                for half in range(2):
                    radp = ps1.tile([NSP, 8, A], DT.float32, tag="radp",
                                    name=f"radp{cc}{half}")
                    nc.tensor.matmul(radp[:], ohs_sb[:], rtm[:, 8 * half:8 * (half + 1), :],
                                     start=True, stop=True)
                    nc.scalar.copy(radsb[:, :, 8 * half:8 * (half + 1)],
                                   radp[:].rearrange("s z i -> s i z"))
                for s in range(NSP):
                    nc.sync.dma_start(out[cc, :, 16 * s:16 * (s + 1)], radsb[s:s + 1])

                # angular
                selI = sbC.tile([A, K], DT.float32, tag="selI", name=f"selI{cc}")
                nc.sync.dma_start(selI[:], selit[cc])
                selJ = sbC.tile([A, K], DT.float32, tag="selJ", name=f"selJ{cc}")
                nc.sync.dma_start(selJ[:], seljt[cc])

                aev = []
                for a in range(4):
                    t = psA.tile([NPAIR_T, 8, A], DT.float32, tag=f"aev{a}",
                                 name=f"aev{cc}_{a}")
                    aev.append(t)

                for ch in range(CH):
                    sl = slice(PCH * ch, PCH * (ch + 1))
                    gp1 = ps1.tile([PCH, 3, A], DT.float32, tag="gp1", name=f"gp1_{cc}_{ch}")
                    nc.tensor.matmul(gp1[:], selI[:, sl], geo[:], start=True, stop=True)
                    gp2 = ps1.tile([PCH, 3, A], DT.float32, tag="gp2", name=f"gp2_{cc}_{ch}")
                    nc.tensor.matmul(gp2[:], selJ[:, sl], geo[:], start=True, stop=True)
                    sb1 = sbK.tile([PCH, 3, A], DT.float32, tag="sb1", name=f"sb1_{cc}_{ch}")
                    nc.scalar.copy(sb1[:], gp1[:])
                    sb2 = sbK.tile([PCH, 3, A], DT.float32, tag="sb2", name=f"sb2_{cc}_{ch}")
                    nc.vector.tensor_copy(sb2[:], gp2[:])
                    d1, dq1, fc1 = sb1[:, 0, :], sb1[:, 1, :], sb1[:, 2, :]
                    d2, dq2, fc2 = sb2[:, 0, :], sb2[:, 1, :], sb2[:, 2, :]

                    jpf = sbK.tile([PCH, A], DT.float32, tag="jpf", name=f"jpf{cc}_{ch}")
                    nc.sync.dma_start(jpf[:], seljpf[cc, sl])
                    jm = sbK.tile([PCH, A], DT.float32, tag="jm", name=f"jm{cc}_{ch}")
                    nc.vector.tensor_tensor(jm[:], dq1, jpf[:], ALU.mult)
                    djk2 = sbK.tile([PCH, 1], DT.float32, tag="djk2", name=f"djk2{cc}_{ch}")
                    nc.vector.tensor_reduce(djk2[:], jm[:], mybir.AxisListType.X, ALU.add)

                    tsum = sbK.tile([PCH, A], DT.float32, tag="tsum", name=f"ts{cc}_{ch}")
                    nc.gpsimd.tensor_tensor(tsum[:], d1, d2, ALU.add)
                    nsum = sbK.tile([PCH, A], DT.float32, tag="nsum", name=f"ns{cc}_{ch}")
                    nc.gpsimd.tensor_tensor(nsum[:], dq1, dq2, ALU.add)
                    den = sbK.tile([PCH, A], DT.float32, tag="den", name=f"den{cc}_{ch}")
                    nc.gpsimd.tensor_tensor(den[:], d1, d2, ALU.mult)

                    nn = sbK.tile([PCH, A], DT.float32, tag="nn", name=f"nn{cc}_{ch}")
                    nc.vector.tensor_scalar(nn[:], nsum[:], djk2[:, 0:1], 0.5,
                                            ALU.subtract, ALU.mult)
                    lden = sbK.tile([PCH, A], DT.float32, tag="lden", name=f"ld{cc}_{ch}")
                    nc.scalar.activation(lden[:], den[:], AF.Ln)
                    rcp = sbK.tile([PCH, A], DT.float32, tag="rcp", name=f"rcp{cc}_{ch}")
                    nc.scalar.activation(rcp[:], lden[:], AF.Exp, scale=-1.0)
                    u = sbK.tile([PCH, A], DT.float32, tag="u", name=f"u{cc}_{ch}")
                    nc.vector.tensor_tensor(u[:], nn[:], rcp[:], ALU.mult)
                    nc.vector.tensor_scalar(u[:], u[:], 1.0, -1.0, ALU.min, ALU.max)
                    usq = sbK.tile([PCH, A], DT.float32, tag="usq", name=f"usq{cc}_{ch}")
                    nc.scalar.square(usq[:], u[:])
                    ssq = sbK.tile([PCH, A], DT.float32, tag="ssq", name=f"ssq{cc}_{ch}")
                    nc.vector.tensor_scalar(ssq[:], usq[:], -0.9025, 1.0, ALU.mult, ALU.add)
                    lss = sbK.tile([PCH, A], DT.float32, tag="lss", name=f"lss{cc}_{ch}")
                    nc.scalar.activation(lss[:], ssq[:], AF.Ln)
                    ss = sbK.tile([PCH, A], DT.float32, tag="ss", name=f"ss{cc}_{ch}")
                    nc.scalar.activation(ss[:], lss[:], AF.Exp, scale=0.5)

                    g = sbK.tile([PCH, A], DT.float32, tag="g", name=f"g{cc}_{ch}")
                    nc.vector.tensor_tensor(g[:], fc1, fc2, ALU.mult)

                    f2 = sbK.tile([PCH, 4, A], DT.float32, tag="f2", name=f"f2{cc}_{ch}")
                    for a in range(4):
                        nc.scalar.activation(f2[:, a, :], tsum[:], AF.Square,
                                             bias=cst(-2.0 * SHF_A[a])[:PCH, 0:1])
                    nc.scalar.activation(f2[:], f2[:], AF.Exp, scale=-2.0)
                    f2g = sbK.tile([PCH, 4, A], DT.bfloat16, tag="f2g", name=f"f2g{cc}_{ch}")
                    nc.vector.tensor_tensor(
                        f2g[:], f2[:],
                        g[:].rearrange("p (a i) -> p a i", a=1).broadcast_to([PCH, 4, A]),
                        ALU.mult)

                    h = sbK.tile([PCH, 8, A], DT.float32, tag="h", name=f"h{cc}_{ch}")
                    for z in range(8):
                        nc.scalar.activation(h[:, z, :], u[:], AF.Copy,
                                             bias=0.5, scale=0.475 * CZ[z])
                        t2 = sbK.tile([PCH, A], DT.float32, tag="t2", name=f"t2{cc}_{ch}_{z}")
                        nc.vector.tensor_scalar(t2[:], ss[:], 0.5 * SZ[z], None, ALU.mult)
                        nc.gpsimd.tensor_tensor(h[:, z, :], h[:, z, :], t2[:], ALU.add)
                    nc.vector.tensor_scalar(h[:], h[:], 0.0, None, ALU.max)
                    lnh = sbK.tile([PCH, 8, A], DT.float32, tag="lnh", name=f"lnh{cc}_{ch}")
                    nc.scalar.activation(lnh[:], h[:], AF.Ln)
                    f1 = sbK.tile([PCH, 8, A], DT.bfloat16, tag="f1", name=f"f1{cc}_{ch}")
                    nc.scalar.activation(f1[:], lnh[:], AF.Exp, scale=ZETA)

                    at = sbK.tile([PCH, 4, 8, A], DT.bfloat16, tag="at", name=f"at{cc}_{ch}")
                    for a in range(4):
                        nc.vector.tensor_tensor(
                            at[:, a], f1[:],
                            f2g[:, a:a + 1, :].broadcast_to([PCH, 8, A]),
                            ALU.mult)

                    ohp_sb = sbK.tile([PCH, NPAIR_T], DT.bfloat16, tag="ohp",
                                      name=f"ohp{cc}_{ch}")
                    nc.sync.dma_start(ohp_sb[:], ohp[cc, sl])
                    for a in range(4):
                        nc.tensor.matmul(aev[a][:], ohp_sb[:], at[:, a],
                                         start=(ch == 0), stop=(ch == CH - 1))

                aevsb = sbC.tile([NPAIR_T, A, 32], DT.float32, tag="aevsb",
                                 name=f"aevsb{cc}")
                for a in range(4):
                    nc.scalar.copy(aevsb[:, :, 8 * a:8 * (a + 1)],
                                   aev[a][:].rearrange("t z i -> t i z"))
                for t in range(NPAIR_T):
                    nc.sync.dma_start(out[cc, :, A + 32 * t:A + 32 * (t + 1)], aevsb[t:t + 1])

    _split_multiwaits(nc)
    return nc


# ---------------- host side ----------------

def _prep(species, coordinates):
    sp = np.asarray(species).astype(np.int64)
    co = np.ascontiguousarray(np.asarray(coordinates), dtype=np.float32)
    d2 = ((co[:, :, None, :].astype(np.float64) - co[:, None, :, :]) ** 2).sum(-1)
    D = np.sqrt(d2)
    for c in range(C):
        np.fill_diagonal(D[c], 1e9)
    near = D < (RCA + 0.02)                       # (C, A, A)
    live = (near[:, :, IDX_I] & near[:, :, IDX_J]).any(axis=1)   # (C, P)
    maxlive = int(live.sum(axis=1).max())
    K = min(P_FULL, max(PCH, int(np.ceil(maxlive / PCH)) * PCH))

    order = np.argsort(~live, axis=1, kind="stable")[:, :K]      # live first
    Isel = IDX_I[order]                            # (C, K)
    Jsel = IDX_J[order]
    lmask = np.take_along_axis(live, order, axis=1)

    ar = np.arange(A)
    selit = (ar[None, :, None] == Isel[:, None, :]).astype(np.float32)   # (C, A, K)
    seljt = (ar[None, :, None] == Jsel[:, None, :]).astype(np.float32)
    seljpf = (ar[None, None, :] == Jsel[:, :, None]).astype(np.float32)  # (C, K, A)
    pid = _tbl[sp[np.arange(C)[:, None], Isel], sp[np.arange(C)[:, None], Jsel]]
    ohp = (2.0 * (pid[:, :, None] == np.arange(NPAIR_T)) * lmask[:, :, None]
           ).astype(ml_dtypes.bfloat16)                                   # (C, K, 10)
    ohs = (sp[:, :, None] == np.arange(NSP)).astype(ml_dtypes.bfloat16)  # (C, A, 4)
    return K, co, selit, seljt, seljpf, ohp, ohs


def _run(species, coordinates, trace=False):
    K, co, selit, seljt, seljpf, ohp, ohs = _prep(species, coordinates)
    if K not in _NC_CACHE:
        _NC_CACHE[K] = _build(K)
    nc = _NC_CACHE[K]
    in_maps = []
    for k in range(NCORES):
        sl = slice(CPC * k, CPC * (k + 1))
        in_maps.append({
            "coords": np.ascontiguousarray(co[sl]),
            "selit": np.ascontiguousarray(selit[sl]),
            "seljt": np.ascontiguousarray(seljt[sl]),
            "seljpf": np.ascontiguousarray(seljpf[sl]),
            "ohp": np.ascontiguousarray(ohp[sl]),
            "ohs": np.ascontiguousarray(ohs[sl]),
        })
    res = run_bass_kernel_spmd(nc, in_maps, core_ids=list(range(NCORES)), trace=trace)
    outs = np.concatenate([res.results[k]["out"] for k in range(NCORES)], axis=0)
    return outs.astype(np.float32), res


def kernel(species, coordinates):
    out, _ = _run(species, coordinates, trace=False)
    return out


# revision 16
# speedup vs baseline: 1.1478x; 1.1478x over previous
"""ANI-style AEVComputer on 8 TRN2 NeuronCores (Bass/Tile).

Strategy
--------
Data-parallel over conformations: each of the 8 cores processes 2 of the 16
conformations end to end; no cross-core communication.

Per conformation, on device:
  *  d^2 matrix via one 5-wide TensorE matmul  (A@B^T with A=[c,q,1], B=[-2c,1,q])
  *  d = exp(ln(d2)/2), cutoff fns via ACT Sin (arg folded into [-pi,pi])
  *  radial AEV: 16 shifted gaussians * fc, scattered over species by matmul
  *  angular AEV over atom pairs (j,k): all pair geometry is derived from the
     d/d^2/fc matrices by selection matmuls (rows I[p], J[p]); the
     (1+cos(theta-shfz))/2 ** 32 factor uses cos(theta-shfz) = c*cz + s*sz and
     pow via exp(32*ln(h)); scatter over the 10 species-pair bins by matmul
     with per-pair one-hot weights (PSUM-accumulated across pair chunks).

The host precomputes integer-derived selection/one-hot tables only (species
one-hots, pair index one-hots); all floating-point geometry math runs on
device.  Pairs (j,k) that can't contribute (no atom within the angular cutoff
of both) are compacted out on host; the kernel is compiled for the padded
live-pair count K (cached per K).
"""
import sys

if '/opt/trn_rl_repo' not in sys.path:
    sys.path.insert(0, '/opt/trn_rl_repo')

import numpy as np
import ml_dtypes

import concourse.bass as bass
import concourse.tile as tile
from concourse import mybir
from concourse.bass_utils import run_bass_kernel_spmd

DT = mybir.dt
AF = mybir.ActivationFunctionType
ALU = mybir.AluOpType

# ---------------- walrus compat: one sync wait per instruction ----------------


def _split_multiwaits(nc):
    n = 0
    for f in nc.m.functions:
        for bb in f.blocks:
            insts = bb.instructions
            out = []
            changed = False
            for inst in insts:
                si = inst.sync_info
                waits = list(si.on_wait) if si is not None else []
                if len(waits) > 1:
                    changed = True
                    for w in waits[:-1]:
                        n += 1
                        out.append(mybir.InstNoOp(
                            name=f"mwsplit-{n}", engine=inst.engine, ins=[], outs=[],
                            sync_info=mybir.SyncInfo(on_wait=[w], on_update=[]),
                        ))
                    inst.sync_info = mybir.SyncInfo(
                        on_wait=[waits[-1]], on_update=list(si.on_update))
                out.append(inst)
            if changed:
                insts.clear()
                insts.extend(out)
    return n


# ---------------- problem constants ----------------
RCR, RCA = 5.2, 3.5
ETA_R, ETA_A, ZETA = 16.0, 8.0, 32.0
SHF_R = (0.9 + 0.26875 * np.arange(16)).astype(np.float64)
SHF_A = np.array([0.9, 1.55, 2.2, 2.85], np.float64)
SHF_Z = (np.pi / 16 + (np.pi / 8) * np.arange(8)).astype(np.float64)
NSP = 4
C, A = 16, 64
NCORES, CPC = 8, 2            # cores, conformations per core
IDX_I, IDX_J = np.triu_indices(A, k=1)
P_FULL = IDX_I.size           # 2016
PCH = 126                     # pairs per chunk (partition dim of chunk tiles)

_tbl = np.zeros((NSP, NSP), np.int64)
_k = 0
for _a in range(NSP):
    for _b in range(_a, NSP):
        _tbl[_a, _b] = _tbl[_b, _a] = _k
        _k += 1
NPAIR_T = _k                  # 10

DIAG = RCR + 1.0              # value added on the diagonal of d

_NC_CACHE = {}
DEBUG_DUMP = False


def _build(K):
    """Build the per-core Bass graph for padded pair count K (multiple of PCH)."""
    CH = K // PCH
    nc = bass.Bass("TRN2", target_bir_lowering=False, debug=False)

    coords = nc.declare_dram_parameter("coords", [CPC, A, 3], DT.float32, isOutput=False)
    selit = nc.declare_dram_parameter("selit", [CPC, A, K], DT.float32, isOutput=False)
    seljt = nc.declare_dram_parameter("seljt", [CPC, A, K], DT.float32, isOutput=False)
    seljpf = nc.declare_dram_parameter("seljpf", [CPC, K, A], DT.float32, isOutput=False)
    ohp = nc.declare_dram_parameter("ohp", [CPC, K, NPAIR_T], DT.bfloat16, isOutput=False)
    ohs = nc.declare_dram_parameter("ohs", [CPC, A, NSP], DT.bfloat16, isOutput=False)
    out = nc.declare_dram_parameter("out", [CPC, A, 384], DT.float32, isOutput=True)
    dbg = None
    if DEBUG_DUMP:
        dbg = nc.declare_dram_parameter("dbg", [CPC, A, 4, A], DT.float32, isOutput=True)

    CZ = np.cos(SHF_Z)
    SZ = np.sin(SHF_Z)

    with tile.TileContext(nc) as tc:
        with tc.tile_pool(name="cpool", bufs=1) as cpool, \
             tc.tile_pool(name="sbC", bufs=2) as sbC, \
             tc.tile_pool(name="sbK", bufs=2) as sbK, \
             tc.tile_pool(name="ps1", bufs=1, space="PSUM") as ps1, \
             tc.tile_pool(name="psA", bufs=1, space="PSUM") as psA:

            consts = {}

            def cst(val):
                v = float(val)
                if v not in consts:
                    t = cpool.tile([128, 1], DT.float32, tag=f"cst{len(consts)}",
                                   name=f"cst{len(consts)}")
                    nc.vector.memset(t[:], v)
                    consts[v] = t
                return consts[v]

            # eye * DIAG via iota + is_equal
            iP = cpool.tile([A, A], DT.float32)
            nc.gpsimd.iota(iP[:], [[0, A]], channel_multiplier=1,
                           allow_small_or_imprecise_dtypes=True)
            iF = cpool.tile([A, A], DT.float32)
            nc.gpsimd.iota(iF[:], [[1, A]], channel_multiplier=0,
                           allow_small_or_imprecise_dtypes=True)
            eye = cpool.tile([A, A], DT.float32)
            nc.vector.tensor_tensor(eye[:], iP[:], iF[:], ALU.is_equal)
            nc.vector.tensor_scalar(eye[:], eye[:], DIAG, None, ALU.mult)
            onesrow = cpool.tile([1, A], DT.float32)
            nc.vector.memset(onesrow[:], 1.0)
            # constant tiles with the shift values expanded along the free dim,
            # so per-shift activation calls collapse into one big op
            czq = cpool.tile([PCH, 8, A], DT.float32)
            szq = cpool.tile([PCH, 8, A], DT.float32)
            for z in range(8):
                nc.vector.memset(czq[:, z, :], 0.475 * CZ[z])
                nc.vector.memset(szq[:, z, :], 0.5 * SZ[z])
            shfa2q = cpool.tile([PCH, 4, A], DT.float32)
            for a in range(4):
                nc.vector.memset(shfa2q[:, a, :], 2.0 * SHF_A[a])
            shfrq = cpool.tile([A, 16, A], DT.float32)
            for r in range(16):
                nc.vector.memset(shfrq[:, r, :], SHF_R[r])

            geos, fcRqs = [], []

            # ---------------- pass 1: per-conformation geometry ----------------
            for cc in range(CPC):
                # A9 = [c^2 ; 1 ; c],  B9 = [1 ; c^2 ; -2c]  ->  A9.T @ B9 = d^2
                A9 = sbC.tile([9, A], DT.float32, tag="A9", name=f"A9{cc}")
                B9 = sbC.tile([9, A], DT.float32, tag="B9", name=f"B9{cc}")
                ct = sbC.tile([3, A], DT.float32, tag="ct", name=f"ct{cc}")
                nc.sync.dma_start(ct[:], coords[cc].rearrange("a k -> k a"))
                nc.scalar.square(A9[0:3, :], ct[:])          # c^2   (parts 0-2)
                nc.vector.memset(B9[0:3, :], 1.0)             # ones (parts 0-2)
                m2ct = sbC.tile([3, A], DT.float32, tag="m2ct", name=f"m2ct{cc}")
                nc.vector.tensor_scalar(m2ct[:], ct[:], -2.0, None, ALU.mult)
                nc.sync.dma_start(A9[3:6, :], B9[0:3, :])     # ones
                nc.sync.dma_start(A9[6:9, :], coords[cc].rearrange("a k -> k a"))
                nc.sync.dma_start(B9[3:6, :], A9[0:3, :])     # c^2
                nc.sync.dma_start(B9[6:9, :], m2ct[:])        # -2c

                dsqp = ps1.tile([A, A], DT.float32, tag="dsq", name=f"dsq{cc}")
                nc.tensor.matmul(dsqp[:], A9[:], B9[:], start=True, stop=True)

                geo = sbC.tile([A, 3, A], DT.float32, tag="geo", name=f"geo{cc}")
                d_t, dsqc, fcA = geo[:, 0, :], geo[:, 1, :], geo[:, 2, :]
                nc.vector.tensor_scalar(dsqc, dsqp[:], 0.0, None, ALU.max)
                lnd = sbC.tile([A, A], DT.float32, tag="lnd", name=f"lnd{cc}")
                nc.scalar.activation(lnd[:], dsqc, AF.Ln)
                nc.scalar.activation(d_t, lnd[:], AF.Exp, scale=0.5)
                nc.vector.tensor_tensor(d_t, d_t, eye[:], ALU.add)

                # cutoffs: fc = mask * (0.5 + 0.5*sin(pi/2 - pi*d/rc))
                fcRq = sbC.tile([A, A], DT.float32, tag="fcRq", name=f"fcRq{cc}")
                for (dst, rc, s1, s2) in ((fcA, RCA, 0.5, 0.5), (fcRq, RCR, 0.125, 0.125)):
                    dcl = sbC.tile([A, A], DT.float32, tag="dcl", name=f"dcl{cc}{rc}")
                    nc.vector.tensor_scalar(dcl[:], d_t, rc * 1.01, None, ALU.min)
                    sn = sbC.tile([A, A], DT.float32, tag="sn", name=f"sn{cc}{rc}")
                    nc.scalar.activation(sn[:], dcl[:], AF.Sin,
                                         bias=cst(np.pi / 2)[:A, 0:1], scale=-np.pi / rc)
                    msk = sbC.tile([A, A], DT.float32, tag="msk", name=f"msk{cc}{rc}")
                    nc.vector.tensor_scalar(msk[:], d_t, rc, None, ALU.is_le)
                    nc.vector.tensor_scalar(sn[:], sn[:], s1, s2, ALU.mult, ALU.add)
                    nc.vector.tensor_tensor(dst, sn[:], msk[:], ALU.mult)

                if DEBUG_DUMP:
                    nc.sync.dma_start(dbg[cc, :, 0:3, :], geo[:])
                    nc.sync.dma_start(dbg[cc, :, 3:4, :],
                                      fcRq[:].rearrange("p (o i) -> p o i", o=1))
                geos.append(geo)
                fcRqs.append(fcRq)

            # ---------------- pass 2: radial + angular ----------------
            for cc in range(CPC):
                geo, fcRq = geos[cc], fcRqs[cc]
                d_t, dsqc, fcA = geo[:, 0, :], geo[:, 1, :], geo[:, 2, :]

                # radial
                rt = sbC.tile([A, 16, A], DT.float32, tag="rt", name=f"rt{cc}")
                nc.vector.tensor_tensor(
                    rt[:], geo[:, 0:1, :].broadcast_to([A, 16, A]), shfrq[:],
                    ALU.subtract)
                nc.scalar.activation(rt[:], rt[:], AF.Square)
                nc.scalar.activation(rt[:], rt[:], AF.Exp, scale=-ETA_R)
                rtm = sbC.tile([A, 16, A], DT.bfloat16, tag="rtm", name=f"rtm{cc}")
                nc.vector.tensor_tensor(
                    rtm[:], rt[:],
                    fcRq[:].rearrange("p (r i) -> p r i", r=1).broadcast_to([A, 16, A]),
                    ALU.mult)
                ohs_sb = sbC.tile([A, NSP], DT.bfloat16, tag="ohs", name=f"ohs{cc}")
                nc.sync.dma_start(ohs_sb[:], ohs[cc])
                radsb = sbC.tile([NSP, A, 16], DT.float32, tag="radsb", name=f"radsb{cc}")
                for half in range(2):
                    radp = ps1.tile([NSP, 8, A], DT.float32, tag="radp",
                                    name=f"radp{cc}{half}")
                    nc.tensor.matmul(radp[:], ohs_sb[:], rtm[:, 8 * half:8 * (half + 1), :],
                                     start=True, stop=True)
                    dst = radsb[:, :, 8 * half:8 * (half + 1)]
                    src = radp[:].rearrange("s z i -> s i z")
                    if half == 0:
                        nc.scalar.copy(dst, src)
                    else:
                        nc.vector.tensor_copy(dst, src)
                for s in range(NSP):
                    nc.sync.dma_start(out[cc, :, 16 * s:16 * (s + 1)], radsb[s:s + 1])

                # angular
                selI = sbC.tile([A, K], DT.float32, tag="selI", name=f"selI{cc}")
                nc.sync.dma_start(selI[:], selit[cc])
                selJ = sbC.tile([A, K], DT.float32, tag="selJ", name=f"selJ{cc}")
                nc.sync.dma_start(selJ[:], seljt[cc])

                aev = []
                for a in range(4):
                    t = psA.tile([NPAIR_T, 8, A], DT.float32, tag=f"aev{a}",
                                 name=f"aev{cc}_{a}")
                    aev.append(t)

                for ch in range(CH):
                    sl = slice(PCH * ch, PCH * (ch + 1))
                    gp1 = ps1.tile([PCH, 3, A], DT.float32, tag="gp1", name=f"gp1_{cc}_{ch}")
                    nc.tensor.matmul(gp1[:], selI[:, sl], geo[:], start=True, stop=True)
                    gp2 = ps1.tile([PCH, 3, A], DT.float32, tag="gp2", name=f"gp2_{cc}_{ch}")
                    nc.tensor.matmul(gp2[:], selJ[:, sl], geo[:], start=True, stop=True)
                    sb1 = sbK.tile([PCH, 3, A], DT.float32, tag="sb1", name=f"sb1_{cc}_{ch}")
                    nc.scalar.copy(sb1[:], gp1[:])
                    sb2 = sbK.tile([PCH, 3, A], DT.float32, tag="sb2", name=f"sb2_{cc}_{ch}")
                    nc.vector.tensor_copy(sb2[:], gp2[:])
                    d1, dq1, fc1 = sb1[:, 0, :], sb1[:, 1, :], sb1[:, 2, :]
                    d2, dq2, fc2 = sb2[:, 0, :], sb2[:, 1, :], sb2[:, 2, :]

                    jpf = sbK.tile([PCH, A], DT.float32, tag="jpf", name=f"jpf{cc}_{ch}")
                    nc.sync.dma_start(jpf[:], seljpf[cc, sl])
                    jm = sbK.tile([PCH, A], DT.float32, tag="jm", name=f"jm{cc}_{ch}")
                    djk2 = sbK.tile([PCH, 1], DT.float32, tag="djk2", name=f"djk2{cc}_{ch}")
                    nc.vector.tensor_tensor(jm[:], dq1, jpf[:], ALU.mult)
                    nc.vector.tensor_reduce(djk2[:], jm[:], mybir.AxisListType.X, ALU.add)

                    tsum = sbK.tile([PCH, A], DT.float32, tag="tsum", name=f"ts{cc}_{ch}")
                    nc.gpsimd.tensor_tensor(tsum[:], d1, d2, ALU.add)
                    nsum = sbK.tile([PCH, A], DT.float32, tag="nsum", name=f"ns{cc}_{ch}")
                    nc.gpsimd.tensor_tensor(nsum[:], dq1, dq2, ALU.add)
                    # dn holds [den | ssq]; one Ln serves both exp(-ln(den)) and
                    # exp(ln(ssq)/2)
                    dn = sbK.tile([PCH, 2, A], DT.float32, tag="dn", name=f"dn{cc}_{ch}")
                    nc.gpsimd.tensor_tensor(dn[:, 0, :], d1, d2, ALU.mult)

                    nn = sbK.tile([PCH, A], DT.float32, tag="nn", name=f"nn{cc}_{ch}")
                    nc.vector.tensor_scalar(nn[:], nsum[:], djk2[:, 0:1], 0.5,
                                            ALU.subtract, ALU.mult)
                    lden = sbK.tile([PCH, A], DT.float32, tag="ld", name=f"ld{cc}_{ch}")
                    nc.scalar.activation(lden[:], dn[:, 0, :], AF.Ln)
                    rcp = sbK.tile([PCH, A], DT.float32, tag="rcp", name=f"rcp{cc}_{ch}")
                    nc.scalar.activation(rcp[:], lden[:], AF.Exp, scale=-1.0)
                    u = sbK.tile([PCH, A], DT.float32, tag="u", name=f"u{cc}_{ch}")
                    nc.vector.tensor_tensor(u[:], nn[:], rcp[:], ALU.mult)
                    nc.vector.tensor_scalar(u[:], u[:], 1.0, -1.0, ALU.min, ALU.max)
                    usq = sbK.tile([PCH, A], DT.float32, tag="usq", name=f"usq{cc}_{ch}")
                    nc.vector.tensor_tensor(usq[:], u[:], u[:], ALU.mult)
                    nc.vector.tensor_scalar(dn[:, 1, :], usq[:], -0.9025, 1.0,
                                            ALU.mult, ALU.add)
                    lss = sbK.tile([PCH, A], DT.float32, tag="lss", name=f"lss{cc}_{ch}")
                    nc.scalar.activation(lss[:], dn[:, 1, :], AF.Ln)
                    ss = sbK.tile([PCH, A], DT.float32, tag="ss", name=f"ss{cc}_{ch}")
                    nc.scalar.activation(ss[:], lss[:], AF.Exp, scale=0.5)

                    g = sbK.tile([PCH, A], DT.float32, tag="g", name=f"g{cc}_{ch}")
                    nc.vector.tensor_tensor(g[:], fc1, fc2, ALU.mult)

                    f2 = sbK.tile([PCH, 4, A], DT.float32, tag="f2", name=f"f2{cc}_{ch}")
                    nc.vector.tensor_tensor(
                        f2[:], tsum[:].rearrange("p (a i) -> p a i", a=1
                                                 ).broadcast_to([PCH, 4, A]),
                        shfa2q[:], ALU.subtract)
                    nc.scalar.activation(f2[:], f2[:], AF.Square)
                    nc.scalar.activation(f2[:], f2[:], AF.Exp, scale=-2.0)
                    f2g = sbK.tile([PCH, 4, A], DT.bfloat16, tag="f2g", name=f"f2g{cc}_{ch}")
                    nc.vector.tensor_tensor(
                        f2g[:], f2[:],
                        g[:].rearrange("p (a i) -> p a i", a=1).broadcast_to([PCH, 4, A]),
                        ALU.mult)

                    h = sbK.tile([PCH, 8, A], DT.float32, tag="h", name=f"h{cc}_{ch}")
                    th2 = sbK.tile([PCH, 8, A], DT.float32, tag="th2", name=f"th2{cc}_{ch}")
                    nc.vector.tensor_tensor(
                        h[:], u[:].rearrange("p (z i) -> p z i", z=1
                                             ).broadcast_to([PCH, 8, A]),
                        czq[:], ALU.mult)
                    nc.gpsimd.tensor_tensor(
                        th2[:], ss[:].rearrange("p (z i) -> p z i", z=1
                                                ).broadcast_to([PCH, 8, A]),
                        szq[:], ALU.mult)
                    nc.gpsimd.tensor_tensor(h[:], h[:], th2[:], ALU.add)
                    nc.vector.tensor_scalar(h[:], h[:], 0.5, 0.0, ALU.add, ALU.max)
                    lnh = sbK.tile([PCH, 8, A], DT.float32, tag="lnh", name=f"lnh{cc}_{ch}")
                    nc.scalar.activation(lnh[:], h[:], AF.Ln)
                    f1 = sbK.tile([PCH, 8, A], DT.bfloat16, tag="f1", name=f"f1{cc}_{ch}")
                    nc.scalar.activation(f1[:], lnh[:], AF.Exp, scale=ZETA)

                    at = sbK.tile([PCH, 4, 8, A], DT.bfloat16, tag="at", name=f"at{cc}_{ch}")
                    for a in range(4):
                        nc.vector.tensor_tensor(
                            at[:, a], f1[:],
                            f2g[:, a:a + 1, :].broadcast_to([PCH, 8, A]),
                            ALU.mult)

                    ohp_sb = sbK.tile([PCH, NPAIR_T], DT.bfloat16, tag="ohp",
                                      name=f"ohp{cc}_{ch}")
                    nc.sync.dma_start(ohp_sb[:], ohp[cc, sl])
                    for a in range(4):
                        nc.tensor.matmul(aev[a][:], ohp_sb[:], at[:, a],
                                         start=(ch == 0), stop=(ch == CH - 1))

                aevsb = sbC.tile([NPAIR_T, A, 32], DT.float32, tag="aevsb",
                                 name=f"aevsb{cc}")
                for a in range(4):
                    dst = aevsb[:, :, 8 * a:8 * (a + 1)]
                    src = aev[a][:].rearrange("t z i -> t i z")
                    if a % 2 == 0:
                        nc.scalar.copy(dst, src)
                    else:
                        nc.vector.tensor_copy(dst, src)
                for t in range(NPAIR_T):
                    nc.sync.dma_start(out[cc, :, A + 32 * t:A + 32 * (t + 1)], aevsb[t:t + 1])

    _split_multiwaits(nc)
    return nc


# ---------------- host side ----------------

def _prep(species, coordinates):
    sp = np.asarray(species).astype(np.int64)
    co = np.ascontiguousarray(np.asarray(coordinates), dtype=np.float32)
    d2 = ((co[:, :, None, :].astype(np.float64) - co[:, None, :, :]) ** 2).sum(-1)
    D = np.sqrt(d2)
    for c in range(C):
        np.fill_diagonal(D[c], 1e9)
    near = D < (RCA + 0.02)                       # (C, A, A)
    live = (near[:, :, IDX_I] & near[:, :, IDX_J]).any(axis=1)   # (C, P)
    maxlive = int(live.sum(axis=1).max())
    K = min(P_FULL, max(PCH, int(np.ceil(maxlive / PCH)) * PCH))

    order = np.argsort(~live, axis=1, kind="stable")[:, :K]      # live first
    Isel = IDX_I[order]                            # (C, K)
    Jsel = IDX_J[order]
    lmask = np.take_along_axis(live, order, axis=1)

    ar = np.arange(A)
    selit = (ar[None, :, None] == Isel[:, None, :]).astype(np.float32)   # (C, A, K)
    seljt = (ar[None, :, None] == Jsel[:, None, :]).astype(np.float32)
    seljpf = (ar[None, None, :] == Jsel[:, :, None]).astype(np.float32)  # (C, K, A)
    pid = _tbl[sp[np.arange(C)[:, None], Isel], sp[np.arange(C)[:, None], Jsel]]
    ohp = (2.0 * (pid[:, :, None] == np.arange(NPAIR_T)) * lmask[:, :, None]
           ).astype(ml_dtypes.bfloat16)                                   # (C, K, 10)
    ohs = (sp[:, :, None] == np.arange(NSP)).astype(ml_dtypes.bfloat16)  # (C, A, 4)
    return K, co, selit, seljt, seljpf, ohp, ohs


def _run(species, coordinates, trace=False):
    K, co, selit, seljt, seljpf, ohp, ohs = _prep(species, coordinates)
    if K not in _NC_CACHE:
        _NC_CACHE[K] = _build(K)
    nc = _NC_CACHE[K]
    in_maps = []
    for k in range(NCORES):
        sl = slice(CPC * k, CPC * (k + 1))
        in_maps.append({
            "coords": np.ascontiguousarray(co[sl]),
            "selit": np.ascontiguousarray(selit[sl]),
            "seljt": np.ascontiguousarray(seljt[sl]),
            "seljpf": np.ascontiguousarray(seljpf[sl]),
            "ohp": np.ascontiguousarray(ohp[sl]),
            "ohs": np.ascontiguousarray(ohs[sl]),
        })
    res = run_bass_kernel_spmd(nc, in_maps, core_ids=list(range(NCORES)), trace=trace)
    outs = np.concatenate([res.results[k]["out"] for k in range(NCORES)], axis=0)
    return outs.astype(np.float32), res


def kernel(species, coordinates):
    out, _ = _run(species, coordinates, trace=False)
    return out
